# revision 7
# baseline (speedup 1.0000x reference)
"""Bidirectional selective-scan SSM (CausalMolSSM) on 8 TRN2 NeuronCores.

Strategy (v3):
  Phase 1 (L-sharded): in-proj, depthwise causal conv, silu, x-proj,
    dt-proj for both directions, feature-major.  dt ships RAW (pre
    softplus) in bf16; B/C are packed into the same AllToAll payload
    (replicated per destination) so each direction needs exactly ONE
    collective sync before phase 2.
  Phase 2 (channel-sharded): softplus via exp->ln activations (one act
    table switch for the whole program), lane broadcasts via bf16
    selection matmuls, dA=exp on Scalar, dBu/yp as bf16 2x vector mults,
    recurrence via tensor_tensor_scan (Vector).  Bwd direction scans with
    REVERSED access patterns - zero data reversals.  GpSimd stays idle
    (SBUF port contention with scans).  Both directions share PSUM pools;
    bwd prep is issued before the fwd epilogue so it overlaps.
  Phase 3: fwd half runs inside the AllToAll#2(bwd) wait window; tail is
    only the bwd gating + out-proj + fusion.
"""
import sys
sys.path.insert(0, '/opt/trn_rl_repo')
import numpy as np
import ml_dtypes

D_MODEL, D_STATE, D_CONV, L = 512, 16, 4, 2048
DI = 1024
NCORES = 8
LC = L // NCORES            # 256
HALO = LC + 6               # 262
NT = 16                     # lane tiles per direction (8 ch x 16 sigma each)
MQ = 512                    # phase-2 free-dim chunk (one PSUM bank)
NQ = L // MQ                # 4
CROWS = 2 * 128 + 32        # AllToAll#1 rows per dst: dt, xc, B/C

BF16 = ml_dtypes.bfloat16


def build_bass():
    import concourse.bass as bass
    import concourse.bacc as bacc
    import concourse.tile as tile
    import concourse.mybir as mybir

    dt = mybir.dt
    Alu = mybir.AluOpType
    Act = mybir.ActivationFunctionType

    nc = bacc.Bacc("TRN2", target_bir_lowering=False, debug=False,
                   enable_asserts=True, num_devices=NCORES)

    f32, f32r, bf = dt.float32, dt.float32r, dt.bfloat16

    # ---------------- DRAM I/O ----------------
    xT = nc.dram_tensor("xT", [D_MODEL, HALO], bf, kind="ExternalInput")
    din = {}
    for d in ("f", "b"):
        din[f"inW_{d}"] = nc.dram_tensor(f"inW_{d}", [D_MODEL, 2 * DI], bf, kind="ExternalInput")
        din[f"xpW_{d}"] = nc.dram_tensor(f"xpW_{d}", [DI, DI + 2 * D_STATE], bf, kind="ExternalInput")
        din[f"dtW_{d}"] = nc.dram_tensor(f"dtW_{d}", [DI, DI], bf, kind="ExternalInput")
        din[f"outW_{d}"] = nc.dram_tensor(f"outW_{d}", [DI, D_MODEL], bf, kind="ExternalInput")
        din[f"inbx_{d}"] = nc.dram_tensor(f"inbx_{d}", [128, 8], f32, kind="ExternalInput")
        din[f"inbz_{d}"] = nc.dram_tensor(f"inbz_{d}", [128, 8], f32, kind="ExternalInput")
        din[f"xpbd_{d}"] = nc.dram_tensor(f"xpbd_{d}", [128, 8], f32, kind="ExternalInput")
        din[f"xpbbc_{d}"] = nc.dram_tensor(f"xpbbc_{d}", [32, 1], f32, kind="ExternalInput")
        din[f"dtb_{d}"] = nc.dram_tensor(f"dtb_{d}", [128, 8], f32, kind="ExternalInput")
        din[f"outb_{d}"] = nc.dram_tensor(f"outb_{d}", [128, 4], f32, kind="ExternalInput")
        din[f"convw_{d}"] = nc.dram_tensor(f"convw_{d}", [128, 32], f32, kind="ExternalInput")
        din[f"convb_{d}"] = nc.dram_tensor(f"convb_{d}", [128, 8], f32, kind="ExternalInput")
    fusW = nc.dram_tensor("fusW", [2 * D_MODEL, D_MODEL], bf, kind="ExternalInput")
    fusb = nc.dram_tensor("fusb", [128, 4], f32, kind="ExternalInput")
    Alan = nc.dram_tensor("Alan", [128, NT], f32, kind="ExternalInput")
    Dpl = nc.dram_tensor("Dpl", [128, 1], f32, kind="ExternalInput")
    OnesT = nc.dram_tensor("OnesT", [128, 1], f32, kind="ExternalInput")
    E128m = nc.dram_tensor("E128m", [128, 16 * 128], bf, kind="ExternalInput")
    E16m = nc.dram_tensor("E16m", [16, 128], bf, kind="ExternalInput")
    SEL128m = nc.dram_tensor("SEL128m", [128, 16 * 128], bf, kind="ExternalInput")
    outT = nc.dram_tensor("outT", [D_MODEL, LC], f32, kind="ExternalOutput")

    RG = [list(range(NCORES))]

    with tile.TileContext(nc) as tc:
        with tc.tile_pool(name="dram", bufs=1, space="DRAM") as dram, \
             tc.tile_pool(name="persist", bufs=1) as pp, \
             tc.tile_pool(name="const", bufs=1) as cp:

            c1_in = [dram.tile([NCORES, CROWS, LC], bf, tag=f"c1in{i}", name=f"c1in{i}")
                     for i in range(2)]
            c1_out = [dram.tile([NCORES, CROWS, LC], bf, tag=f"c1out{i}", name=f"c1out{i}")
                      for i in range(2)]
            c2_in = [dram.tile([NCORES, 1, 128, LC], bf, tag=f"c2in{i}", name=f"c2in{i}")
                     for i in range(2)]
            c2_out = [dram.tile([NCORES, 1, 128, LC], bf, tag=f"c2out{i}", name=f"c2out{i}")
                      for i in range(2)]

            # constants
            e128 = cp.tile([128, 16 * 128], bf, tag="e128")
            e16 = cp.tile([16, 128], bf, tag="e16")
            sel128 = cp.tile([128, 16 * 128], bf, tag="sel128")
            alan = cp.tile([128, NT], f32, tag="alan")
            dpl = cp.tile([128, 1], f32, tag="dpl")
            ones = cp.tile([128, 1], f32, tag="ones")
            nc.sync.dma_start(e128[:], E128m[:])
            nc.sync.dma_start(e16[:], E16m[:])
            nc.sync.dma_start(sel128[:], SEL128m[:])
            nc.sync.dma_start(alan[:], Alan[:])
            nc.sync.dma_start(dpl[:], Dpl[:])
            nc.sync.dma_start(ones[:], OnesT[:])

            # prefetch phase-3 weights early (DMA overlaps phase 1-2)
            p3w = {}
            for d in ("f", "b"):
                ob = pp.tile([128, 4], f32, tag=f"outb{d}")
                nc.sync.dma_start(ob[:], din[f"outb_{d}"][:])
                p3w[("outb", d)] = ob
                for k in range(8):
                    t = pp.tile([128, D_MODEL], bf, tag=f"outw{d}{k}")
                    nc.sync.dma_start(t[:], din[f"outW_{d}"][128 * k:128 * (k + 1), :])
                    p3w[("outw", d, k)] = t
            fbt = pp.tile([128, 4], f32, tag="fusb")
            nc.sync.dma_start(fbt[:], fusb[:])
            fwt = []
            for k in range(8):
                t = pp.tile([128, D_MODEL], bf, tag=f"fw{k}")
                nc.sync.dma_start(t[:], fusW[128 * k:128 * (k + 1), :])
                fwt.append(t)

            zs = {}   # persistent silu(z) tiles, (128, LC) bf16, [dir][m]

            # ================= PHASE 1 (per direction; single AllToAll
            # carrying dt, xc and replicated B/C issued as soon as the
            # payload is staged) =================
            for didx, d in enumerate(("f", "b")):
                off = 0 if d == "f" else 3
                with tc.tile_pool(name=f"p1w_{d}", bufs=1) as wp, \
                     tc.tile_pool(name=f"p1a_{d}", bufs=1) as ap_, \
                     tc.tile_pool(name=f"p1ps_{d}", bufs=4, space="PSUM") as ps1, \
                     tc.tile_pool(name=f"p1sc_{d}", bufs=3) as scp:

                    # biases
                    inbx = scp.tile([128, 8], f32, tag="inbx")
                    inbz = scp.tile([128, 8], f32, tag="inbz")
                    xpbd = scp.tile([128, 8], f32, tag="xpbd")
                    xpbbc = scp.tile([32, 1], f32, tag="xpbbc")
                    dtb = scp.tile([128, 8], f32, tag="dtb")
                    convw = scp.tile([128, 32], f32, tag="convw")
                    convb = scp.tile([128, 8], f32, tag="convb")
                    nc.sync.dma_start(inbx[:], din[f"inbx_{d}"][:])
                    nc.sync.dma_start(inbz[:], din[f"inbz_{d}"][:])
                    nc.sync.dma_start(xpbd[:], din[f"xpbd_{d}"][:])
                    nc.sync.dma_start(xpbbc[:], din[f"xpbbc_{d}"][:])
                    nc.sync.dma_start(dtb[:], din[f"dtb_{d}"][:])
                    nc.sync.dma_start(convw[:], din[f"convw_{d}"][:])
                    nc.sync.dma_start(convb[:], din[f"convb_{d}"][:])

                    # x tiles
                    xsb = []
                    for k in range(4):
                        t = ap_.tile([128, HALO], bf, tag=f"x{k}")
                        nc.sync.dma_start(t[:], xT[128 * k:128 * (k + 1), :])
                        xsb.append(t)

                    # in-proj weights
                    inw = []
                    for k in range(4):
                        t = wp.tile([128, 2 * DI], bf, tag=f"inw{k}")
                        nc.sync.dma_start(t[:], din[f"inW_{d}"][128 * k:128 * (k + 1), :])
                        inw.append(t)

                    xs = []     # pre-conv x_ssm tiles (128, HALO) f32
                    for m in range(8):
                        px = ps1.tile([128, HALO], f32, tag="p1")
                        for k in range(4):
                            nc.tensor.matmul(px[:], inw[k][:, 128 * m:128 * (m + 1)],
                                             xsb[k][:], start=(k == 0), stop=(k == 3))
                        t = ap_.tile([128, HALO], f32, tag=f"xs{m}")
                        nc.scalar.activation(t[:], px[:], Act.Identity,
                                             bias=inbx[:, m:m + 1])
                        xs.append(t)

                    # depthwise causal conv (vector) + silu (scalar)
                    xconv = []
                    silu_x = []
                    for m in range(8):
                        a0 = ap_.tile([128, LC], f32, tag="cacc0")
                        nc.vector.tensor_scalar(a0[:], xs[m][:, off:off + LC],
                                                convw[:, 4 * m:4 * m + 1],
                                                convb[:, m:m + 1],
                                                Alu.mult, Alu.add)
                        a1 = ap_.tile([128, LC], f32, tag="cacc1")
                        nc.vector.scalar_tensor_tensor(a1[:], xs[m][:, off + 1:off + 1 + LC],
                                                       convw[:, 4 * m + 1:4 * m + 2], a0[:],
                                                       Alu.mult, Alu.add)
                        a2 = ap_.tile([128, LC], f32, tag="cacc2")
                        nc.vector.scalar_tensor_tensor(a2[:], xs[m][:, off + 2:off + 2 + LC],
                                                       convw[:, 4 * m + 2:4 * m + 3], a1[:],
                                                       Alu.mult, Alu.add)
                        xc = ap_.tile([128, LC], bf, tag=f"xc{m}")
                        nc.vector.scalar_tensor_tensor(xc[:], xs[m][:, off + 3:off + 3 + LC],
                                                       convw[:, 4 * m + 3:4 * m + 4], a2[:],
                                                       Alu.mult, Alu.add)
                        xconv.append(xc)
                        sx = ap_.tile([128, LC], bf, tag=f"sx{m}")
                        nc.scalar.activation(sx[:], xc[:], Act.Silu)
                        silu_x.append(sx)
                        nc.sync.dma_start(c1_in[didx][m, 128:256, :], xc[:])

                    # x-proj
                    xpw = []
                    for k in range(8):
                        t = wp.tile([128, DI + 2 * D_STATE], bf, tag=f"xpw{k}")
                        nc.sync.dma_start(t[:], din[f"xpW_{d}"][128 * k:128 * (k + 1), :])
                        xpw.append(t)
                    delta = []
                    for m in range(9):
                        rows = 128 if m < 8 else 32
                        px = ps1.tile([128, LC], f32, tag="p1")
                        for k in range(8):
                            nc.tensor.matmul(px[:rows, :],
                                             xpw[k][:, 128 * m:128 * m + rows],
                                             silu_x[k][:], start=(k == 0), stop=(k == 7))
                        if m < 8:
                            t = ap_.tile([128, LC], bf, tag=f"dl{m}")
                            nc.scalar.activation(t[:], px[:], Act.Identity,
                                                 bias=xpbd[:, m:m + 1])
                            delta.append(t)
                        else:
                            bct = ap_.tile([32, LC], bf, tag="bc")
                            nc.scalar.activation(bct[:], px[:32, :], Act.Identity,
                                                 bias=xpbbc[:])
                            for dst in range(NCORES):
                                nc.sync.dma_start(c1_in[didx][dst, 256:CROWS, :], bct[:])

                    # dt-proj (raw, softplus happens in phase 2)
                    dtw = []
                    for k in range(8):
                        t = wp.tile([128, DI], bf, tag=f"dtw{k}")
                        nc.sync.dma_start(t[:], din[f"dtW_{d}"][128 * k:128 * (k + 1), :])
                        dtw.append(t)
                    for m in range(8):
                        px = ps1.tile([128, LC], f32, tag="p1")
                        for k in range(8):
                            nc.tensor.matmul(px[:], dtw[k][:, 128 * m:128 * (m + 1)],
                                             delta[k][:], start=(k == 0), stop=(k == 7))
                        dts = ap_.tile([128, LC], bf, tag=f"dts{m}")
                        nc.scalar.activation(dts[:], px[:], Act.Identity,
                                             bias=dtb[:, m:m + 1])
                        nc.sync.dma_start(c1_in[didx][m, 0:128, :], dts[:])

                    # the single collective for this direction
                    nc.gpsimd.collective_compute(
                        "AllToAll", Alu.bypass, replica_groups=RG,
                        ins=[c1_in[didx][:].opt()], outs=[c1_out[didx][:].opt()])

                    # z projection (needed only in phase 3)
                    for m in range(8, 16):
                        px = ps1.tile([128, HALO], f32, tag="p1")
                        for k in range(4):
                            nc.tensor.matmul(px[:], inw[k][:, 128 * m:128 * (m + 1)],
                                             xsb[k][:], start=(k == 0), stop=(k == 3))
                        zt = pp.tile([128, LC], bf, tag=f"z{d}{m - 8}")
                        nc.scalar.activation(zt[:], px[:, 3:3 + LC], Act.Silu,
                                             bias=inbz[:, m - 8:m - 7])
                        zs[(d, m - 8)] = zt

            # ================= PHASE 2 (shared pools; bwd prep overlaps
            # fwd epilogue) =================
            with tc.tile_pool(name="p2", bufs=1) as p2, \
                 tc.tile_pool(name="psA", bufs=2, space="PSUM") as psA, \
                 tc.tile_pool(name="psB", bufs=2, space="PSUM") as psB, \
                 tc.tile_pool(name="psY", bufs=1, space="PSUM") as psY, \
                 tc.tile_pool(name="p2t", bufs=3) as tp, \
                 tc.tile_pool(name="p3", bufs=2) as p3, \
                 tc.tile_pool(name="p3c", bufs=1) as p3c:

                def p2_prep(didx):
                    """Load + softplus + broadcasts for direction didx.

                    All tiles except xc_m share tags across directions: the
                    bwd write lands after the fwd last read in program order,
                    so the allocator reuses the same SBUF.
                    """
                    dt_raw = p2.tile([128, L], bf, tag="dtraw")
                    xc_m = p2.tile([128, L], bf, tag=f"xcm{didx}")
                    B_m = p2.tile([16, L], bf, tag="bm")
                    C_m = p2.tile([16, L], bf, tag="cm")
                    nc.sync.dma_start(
                        dt_raw[:].rearrange("p (s c) -> p s c", s=NCORES),
                        c1_out[didx][:, 0:128, :].rearrange("s p c -> p s c"))
                    nc.sync.dma_start(
                        xc_m[:].rearrange("p (s c) -> p s c", s=NCORES),
                        c1_out[didx][:, 128:256, :].rearrange("s p c -> p s c"))
                    nc.sync.dma_start(
                        B_m[:].rearrange("p (s c) -> p s c", s=NCORES),
                        c1_out[didx][:, 256:272, :].rearrange("s p c -> p s c"))
                    nc.sync.dma_start(
                        C_m[:].rearrange("p (s c) -> p s c", s=NCORES),
                        c1_out[didx][:, 272:CROWS, :].rearrange("s p c -> p s c"))
                    # softplus = ln(1 + exp(u)) via Exp then Ln
                    dt_sp = p2.tile([128, L], bf, tag="dtsp")
                    for q in range(NQ):
                        sl = slice(MQ * q, MQ * (q + 1))
                        eu = p2.tile([128, MQ], f32, tag="eu")
                        nc.scalar.activation(eu[:], dt_raw[:, sl], Act.Exp)
                        nc.scalar.activation(dt_sp[:, sl], eu[:], Act.Ln, bias=ones[:])
                    dtx = p2.tile([128, L], bf, tag="dtx")
                    nc.vector.tensor_tensor(dtx[:], dt_sp[:], xc_m[:], Alu.mult)
                    brep = p2.tile([128, L], bf, tag="brep")
                    crep = p2.tile([128, L], bf, tag="crep")
                    for q in range(NQ):
                        sl = slice(MQ * q, MQ * (q + 1))
                        pq = psA.tile([128, MQ], f32, tag="pa")
                        nc.tensor.matmul(pq[:], e16[:], B_m[:, sl],
                                         start=True, stop=True)
                        nc.scalar.activation(brep[:, sl], pq[:], Act.Copy)
                        pq2 = psA.tile([128, MQ], f32, tag="pa")
                        nc.tensor.matmul(pq2[:], e16[:], C_m[:, sl],
                                         start=True, stop=True)
                        nc.scalar.activation(crep[:, sl], pq2[:], Act.Copy)
                    return dt_sp, dtx, xc_m, brep, crep

                def p2_loop(didx, dt_sp, dtx, brep, crep, rev):
                    ypsum = psY.tile([128, L], f32, tag="ypsum")
                    for t in range(NT):
                        dA = tp.tile([128, L], f32, tag="dA")
                        dxr = tp.tile([128, L], bf, tag="dxr")
                        dBu = tp.tile([128, L], bf, tag="dBu")
                        pas = []
                        pbs = []
                        for q in range(NQ):
                            sl = slice(MQ * q, MQ * (q + 1))
                            pa = psA.tile([128, MQ], f32, tag="pa")
                            nc.tensor.matmul(pa[:], e128[:, 128 * t:128 * (t + 1)],
                                             dt_sp[:, sl], start=True, stop=True)
                            pas.append((pa, sl))
                            pb = psB.tile([128, MQ], f32, tag="pb")
                            nc.tensor.matmul(pb[:], e128[:, 128 * t:128 * (t + 1)],
                                             dtx[:, sl], start=True, stop=True)
                            pbs.append((pb, sl))
                        for pa, sl in pas:
                            nc.scalar.activation(dA[:, sl], pa[:], Act.Exp,
                                                 scale=alan[:, t:t + 1])
                        for pb, sl in pbs:
                            nc.scalar.activation(dxr[:, sl], pb[:], Act.Copy)
                        nc.vector.tensor_tensor(dBu[:], dxr[:], brep[:], Alu.mult)
                        h = tp.tile([128, L], bf, tag="h")
                        if rev:
                            nc.vector.tensor_tensor_scan(
                                h[:, ::-1], dA[:, ::-1], dBu[:, ::-1], 0.0,
                                Alu.mult, Alu.add)
                        else:
                            nc.vector.tensor_tensor_scan(
                                h[:], dA[:], dBu[:], 0.0, Alu.mult, Alu.add)
                        yp = tp.tile([128, L], bf, tag="yp")
                        nc.vector.tensor_tensor(yp[:], h[:], crep[:], Alu.mult)
                        for q in range(NQ):
                            sl = slice(MQ * q, MQ * (q + 1))
                            nc.tensor.matmul(ypsum[:, sl],
                                             sel128[:, 128 * t:128 * (t + 1)],
                                             yp[:, sl],
                                             start=(t == 0), stop=(t == NT - 1),
                                             skip_group_check=True)
                    return ypsum

                def p2_epilogue(didx, xc_m, ypsum):
                    y_sb = p2.tile([128, L], bf, tag="ysb")
                    nc.vector.scalar_tensor_tensor(y_sb[:], xc_m[:], dpl[:], ypsum[:],
                                                   Alu.mult, Alu.add)
                    for dst in range(NCORES):
                        nc.sync.dma_start(c2_in[didx][dst, 0, :, :],
                                          y_sb[:, LC * dst:LC * (dst + 1)])
                    nc.gpsimd.collective_compute(
                        "AllToAll", Alu.bypass, replica_groups=RG,
                        ins=[c2_in[didx][:].opt()], outs=[c2_out[didx][:].opt()])

                def p3_half(didx, d):
                    outb = p3w[("outb", d)]
                    gates = []
                    for m in range(8):
                        y3 = p3.tile([128, LC], bf, tag="y3")
                        nc.sync.dma_start(y3[:], c2_out[didx][m, 0, :, :])
                        g = p3.tile([128, LC], bf, tag=f"g{m}")
                        nc.vector.tensor_tensor(g[:], y3[:], zs[(d, m)][:], Alu.mult)
                        gates.append(g)
                    outw = [p3w[("outw", d, k)] for k in range(8)]
                    cat = []
                    for m in range(4):
                        po = psA.tile([128, MQ], f32, tag="pa")
                        for k in range(8):
                            nc.tensor.matmul(po[:, :LC], outw[k][:, 128 * m:128 * (m + 1)],
                                             gates[k][:], start=(k == 0), stop=(k == 7))
                        ct = p3c.tile([128, LC], bf, tag=f"cat{d}{m}")
                        nc.scalar.activation(ct[:], po[:, :LC], Act.Identity,
                                             bias=outb[:, m:m + 1])
                        cat.append(ct)
                    return cat

                # ---- fwd ----
                f_sp, f_dtx, f_xc, f_brep, f_crep = p2_prep(0)
                f_ypsum = p2_loop(0, f_sp, f_dtx, f_brep, f_crep, rev=False)
                # bwd prep issued before fwd epilogue: overlaps the fwd tail
                b_sp, b_dtx, b_xc, b_brep, b_crep = p2_prep(1)
                p2_epilogue(0, f_xc, f_ypsum)
                # ---- bwd ----
                b_ypsum = p2_loop(1, b_sp, b_dtx, b_brep, b_crep, rev=True)
                # fwd half of phase 3 lands in the AllToAll#2(bwd) window
                p2_epilogue(1, b_xc, b_ypsum)
                cat_f = p3_half(0, "f")
                cat_b = p3_half(1, "b")
                cat = cat_f + cat_b

                # fusion
                for m in range(4):
                    pf = psB.tile([128, MQ], f32, tag="pb")
                    for k in range(8):
                        nc.tensor.matmul(pf[:, :LC], fwt[k][:, 128 * m:128 * (m + 1)],
                                         cat[k][:], start=(k == 0), stop=(k == 7))
                    ot = p3.tile([128, LC], f32, tag="ot")
                    nc.scalar.activation(ot[:], pf[:, :LC], Act.Identity, bias=fbt[:, m:m + 1])
                    nc.sync.dma_start(outT[128 * m:128 * (m + 1), :], ot[:])

    nc.compile()
    return nc


def make_in_maps(inputs):
    x = np.asarray(inputs["x"], np.float32)
    A = -np.exp(np.asarray(inputs["A_log"], np.float32))          # (DI, S)
    Dp = np.asarray(inputs["D_param"], np.float32)

    def bias_tiles(b, ntiles):
        return np.ascontiguousarray(
            np.asarray(b, np.float32).reshape(ntiles, 128).T)

    common = {}
    for d, pre in (("f", "fwd_"), ("b", "bwd_")):
        inW = np.asarray(inputs[pre + "in_W"], np.float32)
        inb = np.asarray(inputs[pre + "in_b"], np.float32)
        cw = np.asarray(inputs[pre + "conv_w"], np.float32)
        if d == "b":
            cw = cw[:, ::-1]
        cb = np.asarray(inputs[pre + "conv_b"], np.float32)
        xpW = np.asarray(inputs[pre + "xp_W"], np.float32)
        xpb = np.asarray(inputs[pre + "xp_b"], np.float32)
        dtW = np.asarray(inputs[pre + "dt_W"], np.float32)
        dtb = np.asarray(inputs[pre + "dt_b"], np.float32)
        outW = np.asarray(inputs[pre + "out_W"], np.float32)
        outb = np.asarray(inputs[pre + "out_b"], np.float32)
        common[f"inW_{d}"] = inW.astype(BF16)
        common[f"inbx_{d}"] = bias_tiles(inb[:DI], 8)
        common[f"inbz_{d}"] = bias_tiles(inb[DI:], 8)
        common[f"convw_{d}"] = np.ascontiguousarray(
            cw.reshape(8, 128, 4).transpose(1, 0, 2).reshape(128, 32))
        common[f"convb_{d}"] = bias_tiles(cb, 8)
        common[f"xpW_{d}"] = xpW.astype(BF16)
        common[f"xpbd_{d}"] = bias_tiles(xpb[:DI], 8)
        common[f"xpbbc_{d}"] = np.ascontiguousarray(xpb[DI:].reshape(32, 1))
        common[f"dtW_{d}"] = dtW.astype(BF16)
        common[f"dtb_{d}"] = bias_tiles(dtb, 8)
        common[f"outW_{d}"] = outW.astype(BF16)
        common[f"outb_{d}"] = bias_tiles(outb, 4)
    common["fusW"] = np.asarray(inputs["fusion_W"], np.float32).astype(BF16)
    common["fusb"] = bias_tiles(np.asarray(inputs["fusion_b"], np.float32), 4)
    common["OnesT"] = np.ones((128, 1), np.float32)

    p = np.arange(128)
    e128 = np.zeros((128, 16 * 128), np.float32)
    sel128 = np.zeros((128, 16 * 128), np.float32)
    for t in range(16):
        e128[8 * t + p // 16, 128 * t + p] = 1.0
        sel128[p, 128 * t + 8 * t + p // 16] = 1.0
    e16 = np.zeros((16, 128), np.float32)
    e16[p % 16, p] = 1.0
    common["E128m"] = e128.astype(BF16)
    common["E16m"] = e16.astype(BF16)
    common["SEL128m"] = sel128.astype(BF16)

    in_maps = []
    for c in range(NCORES):
        m = dict(common)
        r0 = LC * c
        xpad = np.zeros((HALO, D_MODEL), np.float32)
        lo, hi = max(0, r0 - 3), min(L, r0 + LC + 3)
        xpad[lo - (r0 - 3): hi - (r0 - 3)] = x[lo:hi]
        m["xT"] = np.ascontiguousarray(xpad.T).astype(BF16)
        A_sh = A[128 * c:128 * (c + 1)]                      # (128, 16)
        m["Alan"] = np.ascontiguousarray(
            A_sh.reshape(16, 8, 16).transpose(1, 2, 0).reshape(128, NT))
        m["Dpl"] = np.ascontiguousarray(Dp[128 * c:128 * (c + 1)].reshape(128, 1))
        in_maps.append(m)
    return in_maps


_CACHE = {}


def kernel(**inputs):
    from concourse.bass_utils import run_bass_kernel_spmd
    if "nc" not in _CACHE:
        _CACHE["nc"] = build_bass()
    nc = _CACHE["nc"]
    in_maps = make_in_maps(inputs)
    res = run_bass_kernel_spmd(nc, in_maps, list(range(NCORES)))
    outs = [res.results[c]["outT"] for c in range(NCORES)]
    full = np.concatenate(outs, axis=1)      # (512, 2048)
    return np.ascontiguousarray(full.T).astype(np.float32)
